# revision 1
# baseline (speedup 1.0000x reference)
"""Trainium2 Bass kernel for a binarized (1w1a) BasicBlock:

    out1 = hardtanh(BN1(binconv(x, w1)))          # BN in training mode (batch stats)
    out  = hardtanh(BN2(binconv(out1, w2)) + x)   # identity shortcut

binconv(x, w) = conv3x3(sign(x), sign(w), pad=1) * (SCALE / K)

Sharding: data-parallel over batch (4 images per core on 8 cores), weights
replicated.  BN batch statistics (per-channel sum and sum-of-squares) are
combined with a tiny cross-core AllReduce.

Implementation notes:
  - sign() values (+-1, 0) are exact in fp8/bf16, and the 3x3x256 conv
    accumulates integers |S| <= 2304 in fp32 PSUM, so the convolutions are
    bit-exact at fp8 TensorE rate.  The SCALE/K factor commutes through
    BatchNorm and is folded into eps:  eps_eff = eps / (SCALE/K)^2.
  - conv3x3 = 9 shifted matmuls accumulated in PSUM, with fp8 DoubleRow
    contracting both 128-channel blocks per pass.  Activations live in SBUF
    as [128 ch-pair, 2, n, 58, 58] zero-padded images, so every shift is an
    access-pattern offset.  Each matmul reads 8 padded rows contiguously
    (8 x 58 = 464 columns); the 2 junk columns between output rows are
    simply never read back.
  - Per-channel statistics ride on ScalarE (activation accum_out fused with
    the PSUM->fp16 copy) and VectorE (fused square+reduce) while TensorE
    streams matmuls.  conv outputs are exact integers in fp16 and get
    re-binarized with a single fused sign(A*y + B) activation per tile.
"""

import numpy as np
import ml_dtypes

import concourse.bass as bass
import concourse.tile as tile
from concourse import bacc, mybir
from concourse import bass_utils

N_CORES = 8
N, C, H, W = 32, 256, 56, 56
NL = N // N_CORES          # images per core
HP = H + 2                 # padded rows (58)
WP = 64                    # padded row pitch (bytes): keeps DoubleRow offsets 16B-aligned
IMG = HP * WP              # 3712 padded image elements
CB = C // 128              # channel blocks (2)
HT = 7                     # output rows per tile
N_HT = H // HT             # 8 tiles per image
FREE = HT * WP             # 448: 7 padded rows read contiguously
SCALE, K = 1.0, 2
EPS = 1e-5
ALPHA = SCALE / K
EPS_EFF = EPS / (ALPHA * ALPHA)
M_TOT = float(N * H * W)   # BN reduction count (global batch)
NL_IMG_STRIDE = IMG        # per-image stride inside a channel block

F32 = mybir.dt.float32
F16 = mybir.dt.float16
BF16 = mybir.dt.bfloat16
FP8 = mybir.dt.float8e4
NP_FP8 = ml_dtypes.float8_e4m3
AF = mybir.ActivationFunctionType
ALU = mybir.AluOpType
DR = mybir.MatmulPerfMode.DoubleRow

_CACHE = {}


def _conv_phase(nc, pools, xbf, wt, y16, recs, copy_eng="scalar"):
    """One binarized conv3x3 over all local images + stats accumulation.

    xbf: [128, 2, NL*IMG] fp8 flat view of padded/binarized inputs
    wt:  [128, 3, 3, 2, C] fp8 weights ([ci, dy, dx, ci_blk, co] layout)
    y16: [cb] list of [128, NL, H, W] f16 outputs (exact integer sums)
    recs: [cb] list of [128, NL * N_HT, 6] f32 bn_stats records (one per
          tile; all tiles have equal counts, as bn_aggr's equal-weight merge
          requires)
    """
    psum = pools["psum"]
    for n in range(NL):
        for ht in range(N_HT):
            h0 = ht * HT
            idx = n * N_HT + ht
            for cob in range(CB):
                # out[h0+r, w] accumulates at pt column r*WP + w + 2 for all
                # nine (dy, dx): the dx shift is applied to the PSUM window so
                # every rhs read stays 16B-aligned (DoubleRow requirement).
                pt = psum.tile([128, FREE + 2], F32, tag="pt", name="pt")
                k = 0
                for dy in range(3):
                    for dx in range(3):
                        off = n * NL_IMG_STRIDE + (h0 + dy) * WP
                        nc.tensor.matmul(
                            pt[:, 2 - dx:FREE + 2 - dx],
                            wt[:, dy, dx, :, cob * 128:(cob + 1) * 128],
                            xbf[:, :, off:off + FREE],
                            start=(k == 0),
                            stop=(k == 8),
                            perf_mode=DR,
                        )
                        k += 1
                ptv = pt[:, 2:FREE + 2].rearrange("p (a b) -> p a b", b=WP)
                ysl = y16[cob][:, n, h0:h0 + HT, :]
                # fp32 PSUM -> exact integers in fp16
                if copy_eng == "scalar":
                    nc.scalar.activation(out=ysl, in_=ptv[:, :, 0:W], func=AF.Copy)
                else:
                    nc.vector.tensor_copy(ysl, ptv[:, :, 0:W])
                # one Welford record per tile (VectorE, contiguous fp16 read)
                yfl = y16[cob][:, n, :, :].rearrange("p a b -> p (a b)")
                nc.vector.bn_stats(
                    out=recs[cob][:, idx, :],
                    in_=yfl[:, h0 * W:(h0 + HT) * W])


def _stats_to_sums(nc, pools, recs, st):
    """bn_aggr per channel block, convert (mean, var) -> (sum, sumsq) local."""
    small = pools["small"]
    m_loc = float(NL * H * W)
    for cob in range(CB):
        mv = small.tile([128, 2], F32, tag=f"mv{cob}", name=f"mv{cob}")
        nc.vector.bn_aggr(out=mv[:], in_=recs[cob][:])
        msq = small.tile([128, 1], F32, tag=f"smsq{cob}", name=f"smsq{cob}")
        nc.vector.tensor_scalar_mul(st[:, 2 * cob:2 * cob + 1], mv[:, 0:1], m_loc)
        nc.vector.tensor_mul(msq[:], mv[:, 0:1], mv[:, 0:1])
        nc.vector.tensor_add(msq[:], msq[:], mv[:, 1:2])
        nc.vector.tensor_scalar_mul(st[:, 2 * cob + 1:2 * cob + 2], msq[:], m_loc)


def _bn_affine(nc, pools, gstats, gb, g_col, b_col, a_out, b_out):
    """Per-channel-block A/B:  A = g * rsqrt(var + eps_eff),  B = b - mean * A.

    gstats: [128, 4] globally-reduced (sum, sumsq) per channel block
    """
    small = pools["small"]
    epst = pools["epst"]
    for cob in range(CB):
        mean = small.tile([128, 1], F32, tag=f"mean{cob}", name=f"mean{cob}")
        ex2 = small.tile([128, 1], F32, tag=f"ex2{cob}", name=f"ex2{cob}")
        msq = small.tile([128, 1], F32, tag=f"msq{cob}", name=f"msq{cob}")
        var = small.tile([128, 1], F32, tag=f"var{cob}", name=f"var{cob}")
        rstd = small.tile([128, 1], F32, tag=f"rstd{cob}", name=f"rstd{cob}")
        nc.vector.tensor_scalar_mul(mean[:], gstats[:, 2 * cob:2 * cob + 1], 1.0 / M_TOT)
        nc.vector.tensor_scalar_mul(ex2[:], gstats[:, 2 * cob + 1:2 * cob + 2], 1.0 / M_TOT)
        # var = ex2 - mean^2
        nc.vector.tensor_mul(msq[:], mean[:], mean[:])
        nc.vector.tensor_sub(var[:], ex2[:], msq[:])
        # rstd = 1 / sqrt(var + eps_eff)
        nc.scalar.activation(out=rstd[:], in_=var[:], func=AF.Sqrt, bias=epst[:])
        nc.vector.reciprocal(rstd[:], rstd[:])
        # A = g * rstd ; B = b - mean * A
        nc.vector.tensor_mul(a_out[cob][:], gb[:, g_col + cob:g_col + cob + 1], rstd[:])
        nc.vector.tensor_mul(mean[:], mean[:], a_out[cob][:])
        nc.vector.tensor_sub(b_out[cob][:], gb[:, b_col + cob:b_col + cob + 1], mean[:])


def build():
    """Build + compile the per-core Bass program (SPMD, 8 cores)."""
    nc = bacc.Bacc("TRN2", target_bir_lowering=False, debug=False,
                   num_devices=N_CORES)

    x_in = nc.dram_tensor("x", [NL, C, H, W], F32, kind="ExternalInput").ap()
    w1_in = nc.dram_tensor("w1t", [3, 3, 128, 2, C], FP8, kind="ExternalInput").ap()
    w2_in = nc.dram_tensor("w2t", [3, 3, 128, 2, C], FP8, kind="ExternalInput").ap()
    gb_in = nc.dram_tensor("gb", [128, 8], F32, kind="ExternalInput").ap()
    out_d = nc.dram_tensor("out", [NL, C, H, W], F32, kind="ExternalOutput").ap()

    rg = [list(range(N_CORES))]

    with tile.TileContext(nc) as tc:
        import contextlib
        with contextlib.ExitStack() as ctx:
            consts = ctx.enter_context(tc.tile_pool(name="consts", bufs=1))
            xbp = ctx.enter_context(tc.tile_pool(name="xbp", bufs=1))
            y16p = ctx.enter_context(tc.tile_pool(name="y16p", bufs=1))
            statp = ctx.enter_context(tc.tile_pool(name="statp", bufs=1))
            small = ctx.enter_context(tc.tile_pool(name="small", bufs=1))
            psum = ctx.enter_context(tc.tile_pool(name="psum", bufs=6, space="PSUM"))
            dram = ctx.enter_context(tc.tile_pool(name="dram", bufs=1, space="DRAM"))
            epst = small.tile([128, 1], F32, tag="epst", name="epst")
            nc.vector.memset(epst[:], EPS_EFF)
            pools = {"psum": psum, "small": small, "epst": epst}

            # ---- dummy AllReduce: absorb first-collective setup cost under conv1
            dzero = small.tile([128, 1], F32, tag="dzero", name="dzero")
            nc.vector.memset(dzero[:], 0.0)
            d_in0 = dram.tile([128, 1], F32, tag="d_in0", name="d_in0")
            d_out0 = dram.tile([128 * N_CORES, 1], F32, tag="d_out0", name="d_out0")
            nc.sync.dma_start(out=d_in0[:], in_=dzero[:])
            nc.gpsimd.collective_compute(
                "AllGather", ALU.bypass, replica_groups=rg,
                ins=[d_in0.opt()], outs=[d_out0.opt()],
            )

            # ---- constants (w1 in its own pool, released after conv1)
            w1p = tc.tile_pool(name="w1p", bufs=1)
            w1pp = w1p.__enter__()
            w1t = w1pp.tile([128, 3, 3, 2, C], FP8, tag="w1t", name="w1t")
            nc.gpsimd.dma_start(
                out=w1t[:],
                in_=w1_in[:].rearrange("dy dx ci two co -> ci dy dx two co"))
            w2t = consts.tile([128, 3, 3, 2, C], FP8, tag="w2t", name="w2t")
            nc.gpsimd.dma_start(
                out=w2t[:],
                in_=w2_in[:].rearrange("dy dx ci two co -> ci dy dx two co"))
            gb = consts.tile([128, 8], F32, tag="gb", name="gb")
            nc.gpsimd.dma_start(out=gb[:], in_=gb_in[:])

            # ---- padded binarized activations (reused: conv1 input, then conv2
            # input).  Rows padded to a 64B pitch so DoubleRow rhs offsets stay
            # 16B-aligned; block stride NL*IMG = 14848 is 16B-aligned too.
            blk = NL * IMG
            assert blk % 16 == 0
            xb = xbp.tile([128, CB, blk], FP8, tag="xb", name="xb")
            xbf = xb[:]
            xbi = [xb[:, cib, :].rearrange(
                "p (n a b) -> p n a b", a=HP, b=WP) for cib in range(CB)]
            # zero only the halo borders + pitch padding (interior is always
            # overwritten by the sign activations before it is read)
            for cib in range(CB):
                nc.vector.memset(xbi[cib][:, :, 0, :], 0.0)
                nc.vector.memset(xbi[cib][:, :, H + 1, :], 0.0)
                nc.vector.memset(xbi[cib][:, :, 1:H + 1, 0:1], 0.0)
                nc.vector.memset(xbi[cib][:, :, 1:H + 1, W + 1:WP], 0.0)

            # ---- conv outputs as exact integers (reused for conv1 then conv2)
            y16 = [y16p.tile([128, NL, H, W], F16, tag=f"y16_{cob}", name=f"y16_{cob}")
                   for cob in range(CB)]

            # ---- bn_stats records
            r1c = [statp.tile([128, NL * N_HT, 6], F32, tag=f"r1c{c}", name=f"r1c{c}") for c in range(CB)]
            r2c = [statp.tile([128, NL * N_HT, 6], F32, tag=f"r2c{c}", name=f"r2c{c}") for c in range(CB)]

            # ---- phase 0: load x, binarize into padded buffers
            with tc.tile_pool(name="stage", bufs=3) as stage:
                dma_rr = [nc.sync]
                for n in range(NL):
                    for cib in range(CB):
                        xs = stage.tile([128, H, W], F32, tag="xstage", name="xstage")
                        dma_rr[0].dma_start(
                            out=xs[:], in_=x_in[n, cib * 128:(cib + 1) * 128, :, :])
                        nc.scalar.activation(
                            out=xbi[cib][:, n, 1:H + 1, 1:W + 1], in_=xs[:],
                            func=AF.Sign)

            # ---- conv1 + stats
            _conv_phase(nc, pools, xbf, w1t, y16, r1c)
            w1p.__exit__(None, None, None)

            # ---- residual prefetch pool: reuses the stage/w1 zones; DMAs run
            # during phase2/conv2 while the DMA engines are otherwise idle
            resp = ctx.enter_context(tc.tile_pool(name="resp", bufs=6))
            youtp = ctx.enter_context(tc.tile_pool(name="youtp", bufs=2))

            # ---- aggregate + AllReduce stats 1
            st1 = small.tile([128, 4], F32, tag="st1", name="st1")
            _stats_to_sums(nc, pools, r1c, st1)
            d_in1 = dram.tile([128, 4], F32, tag="d_in1", name="d_in1")
            d_out1 = dram.tile([128 * N_CORES, 4], F32, tag="d_out1", name="d_out1")
            nc.gpsimd.dma_start(out=d_in1[:], in_=st1[:])
            nc.gpsimd.collective_compute(
                "AllGather", ALU.bypass, replica_groups=rg,
                ins=[d_in1.opt()], outs=[d_out1.opt()],
            )
            gag1 = small.tile([128, 4, N_CORES], F32, tag="gag1", name="gag1")
            nc.gpsimd.dma_start(
                out=gag1[:], in_=d_out1[:].rearrange("(r p) c -> p c r", p=128))
            gstats1 = small.tile([128, 4], F32, tag="gstats1", name="gstats1")
            nc.vector.reduce_sum(gstats1[:], gag1[:], axis=mybir.AxisListType.X)

            a1 = [small.tile([128, 1], F32, tag=f"a1_{c}", name=f"a1_{c}") for c in range(CB)]
            b1 = [small.tile([128, 1], F32, tag=f"b1_{c}", name=f"b1_{c}") for c in range(CB)]
            _bn_affine(nc, pools, gstats1, gb, g_col=0, b_col=2, a_out=a1, b_out=b1)

            # ---- phase 2: out1 = sign(A1 * y1 + B1) into the padded buffers
            for n in range(NL):
                for ht in range(N_HT):
                    h0 = ht * HT
                    for cob in range(CB):
                        nc.scalar.activation(
                            out=xbi[cob][:, n, h0 + 1:h0 + HT + 1, 1:W + 1],
                            in_=y16[cob][:, n, h0:h0 + HT, :],
                            func=AF.Sign,
                            scale=a1[cob][:],
                            bias=b1[cob][:],
                        )

            # ---- residual prefetch (DMA only; overlaps phase2 + conv2)
            xres = []
            dma_rr2 = [nc.sync]
            for n in range(NL):
                for cib in range(CB):
                    xr = resp.tile([128, H, W], F32, tag="xres", name="xres")
                    dma_rr2[0].dma_start(
                        out=xr[:], in_=x_in[n, cib * 128:(cib + 1) * 128, :, :])
                    xres.append(xr)

            # ---- conv2 + stats (y16 overwritten with conv2 integer sums).
            # ScalarE is busy with the phase-2 signs here, so the PSUM copy
            # rides on VectorE instead.
            _conv_phase(nc, pools, xbf, w2t, y16, r2c, copy_eng="vector")

            # ---- aggregate + AllReduce stats 2
            st2 = small.tile([128, 4], F32, tag="st2", name="st2")
            _stats_to_sums(nc, pools, r2c, st2)
            d_in2 = dram.tile([128, 4], F32, tag="d_in2", name="d_in2")
            d_out2 = dram.tile([128 * N_CORES, 4], F32, tag="d_out2", name="d_out2")
            nc.gpsimd.dma_start(out=d_in2[:], in_=st2[:])
            nc.gpsimd.collective_compute(
                "AllGather", ALU.bypass, replica_groups=rg,
                ins=[d_in2.opt()], outs=[d_out2.opt()],
            )
            gag2 = small.tile([128, 4, N_CORES], F32, tag="gag2", name="gag2")
            nc.gpsimd.dma_start(
                out=gag2[:], in_=d_out2[:].rearrange("(r p) c -> p c r", p=128))
            gstats2 = small.tile([128, 4], F32, tag="gstats2", name="gstats2")
            nc.vector.reduce_sum(gstats2[:], gag2[:], axis=mybir.AxisListType.X)

            a2 = [small.tile([128, 1], F32, tag=f"a2_{c}", name=f"a2_{c}") for c in range(CB)]
            b2 = [small.tile([128, 1], F32, tag=f"b2_{c}", name=f"b2_{c}") for c in range(CB)]
            _bn_affine(nc, pools, gstats2, gb, g_col=4, b_col=6, a_out=a2, b_out=b2)

            # ---- final: out = clip(A2 * y2 + B2 + x, -1, 1)
            for n in range(NL):
                for cib in range(CB):
                    xr = xres[n * CB + cib]
                    yout = youtp.tile([128, H, W], F32, tag="yout", name="yout")
                    nc.scalar.activation(
                        out=yout[:], in_=y16[cib][:, n, :, :], func=AF.Identity,
                        scale=a2[cib][:], bias=b2[cib][:])
                    nc.vector.tensor_add(yout[:], yout[:], xr[:])
                    nc.vector.tensor_scalar(
                        out=yout[:], in0=yout[:], scalar1=1.0, scalar2=-1.0,
                        op0=ALU.min, op1=ALU.max)
                    nc.sync.dma_start(
                        out=out_d[n, cib * 128:(cib + 1) * 128, :, :], in_=yout[:])

    nc.compile()
    return nc


def _prep_inputs(x, w1, g1, b1, w2, g2, b2):
    """Host-side sharding + weight layout. Returns per-core input maps."""
    x = np.ascontiguousarray(np.asarray(x, dtype=np.float32))

    # sign(w) in [dy, dx, ci%128, ci//128, co] fp8 layout; +-1/0 exact
    def prep_w(w):
        wt = np.sign(np.asarray(w, np.float32)).transpose(2, 3, 1, 0)  # dy dx ci co
        wt = wt.reshape(3, 3, 2, 128, C).transpose(0, 1, 3, 2, 4)      # dy dx 128 2 co
        return np.ascontiguousarray(wt).astype(NP_FP8)

    w1t = prep_w(w1)
    w2t = prep_w(w2)
    gb = np.stack(
        [np.asarray(v, np.float32)[c * 128:(c + 1) * 128]
         for v in (g1, b1, g2, b2) for c in range(CB)],
        axis=1,
    )
    # column order: g1_0 g1_1 b1_0 b1_1 g2_0 g2_1 b2_0 b2_1
    gb = np.ascontiguousarray(gb)
    in_maps = []
    for c in range(N_CORES):
        in_maps.append({
            "x": x[c * NL:(c + 1) * NL],
            "w1t": w1t,
            "w2t": w2t,
            "gb": gb,
        })
    return in_maps


def run(inputs, trace=False):
    """Run the kernel on 8 cores; returns (full_output, BassKernelResults)."""
    if "nc" not in _CACHE:
        _CACHE["nc"] = build()
    nc = _CACHE["nc"]
    in_maps = _prep_inputs(**inputs)
    res = bass_utils.run_bass_kernel_spmd(
        nc, in_maps, core_ids=list(range(N_CORES)), trace=trace)
    out = np.concatenate([res.results[c]["out"] for c in range(N_CORES)], axis=0)
    return out, res


def kernel(**inputs):
    out, _ = run(inputs, trace=False)
    return out



# revision 2
# speedup vs baseline: 1.1525x; 1.1525x over previous
"""Trainium2 Bass kernel for a binarized (1w1a) BasicBlock:

    out1 = hardtanh(BN1(binconv(x, w1)))          # BN in training mode (batch stats)
    out  = hardtanh(BN2(binconv(out1, w2)) + x)   # identity shortcut

binconv(x, w) = conv3x3(sign(x), sign(w), pad=1) * (SCALE / K)

Sharding: data-parallel over batch (4 images per core on 8 cores), weights
replicated.  BN batch statistics (per-channel sum and sum-of-squares) are
combined with a tiny cross-core AllReduce.

v2 design notes (evolved from the 64-pitch baseline):
  - sign() values (+-1, 0) are exact in fp8, and the 3x3x256 conv accumulates
    integers |S| <= 2304 in fp32 PSUM, so the convolutions are bit-exact at
    fp8 DoubleRow rate.  SCALE/K commutes through BatchNorm and is folded
    into eps: eps_eff = eps / (SCALE/K)^2.
  - conv3x3 = 9 shifted matmuls accumulated in PSUM.  Activations live in
    SBUF as [128, n, cb, 58*58(+12 pad)] zero-haloed images with a 58-elem
    row pitch; every tap is a plain access-pattern offset (HW-verified that
    DoubleRow tolerates unaligned rhs offsets, so no pitch-64 padding and no
    PSUM-window shifting is needed).  Each matmul covers 8 output rows =
    464 streamed columns of which 448 are useful (96.6%).
  - LDWEIGHTS is hidden under MATMUL by the PE's reorder window, so the
    matmul stream runs at ~N/2.4GHz per tap.
  - x is kept resident in SBUF as fp16 for the residual add: no second
    HBM read of x, and the tail is a compute/DMA pipeline.
  - BN stats ride on VectorE (bn_stats per conv tile) while TensorE streams;
    the cross-core exchange is a single small AllReduce per BN.
  - Engine budget: ScalarE does input signs, then the 56 phase-2
    sign(A*y+B) re-binarizations, then the tail affine; VectorE does the
    PSUM->f16 copies, bn_stats, x16 casts, residual adds; GpSimd takes half
    the tail clips; SP queues all data DMA.
"""

import numpy as np
import ml_dtypes

import concourse.bass as bass
import concourse.tile as tile
from concourse import bacc, mybir
from concourse import bass_utils

N_CORES = 8
N, C, H, W = 32, 256, 56, 56
NL = N // N_CORES          # images per core (4)
CB = C // 128              # channel blocks (2)
PITCH = 58                 # padded row pitch (1 halo + 56 + 1 halo)
HP = 58                    # padded rows
IMG_RAW = HP * PITCH       # 3364
IMG = 3376                 # padded to 16B multiple (DoubleRow block stride)
HT = 8                     # output rows per tile
N_HT = H // HT             # 7 tiles per image
FREE = HT * PITCH          # 464 streamed columns per matmul
PIX = H * W                # 3136
SCALE, K = 1.0, 2
EPS = 1e-5
ALPHA = SCALE / K
EPS_EFF = EPS / (ALPHA * ALPHA)
M_TOT = float(N * H * W)   # BN reduction count (global batch)
M_LOC = float(NL * H * W)

F32 = mybir.dt.float32
F16 = mybir.dt.float16
FP8 = mybir.dt.float8e4
NP_FP8 = ml_dtypes.float8_e4m3
AF = mybir.ActivationFunctionType
ALU = mybir.AluOpType
DR = mybir.MatmulPerfMode.DoubleRow

_CACHE = {}


def _conv_phase(nc, pools, xb, wt, y16, y16f, recs, copy_eng):
    """One binarized conv3x3 over all local images + stats accumulation.

    xb:   [128, NL, CB, IMG] fp8 padded/binarized inputs
    wt:   [128, 3, 3, 2, C] fp8 weights ([ci, dy, dx, ci_blk, co] layout)
    y16:  [128, CB, NL, H, W] f16 conv outputs (exact integer sums)
    y16f: [128, CB, NL, PIX] flat view of the same tile
    recs: [128, CB, NL * N_HT, 6] f32 bn_stats records (equal-count tiles,
          as bn_aggr's equal-weight merge requires)
    """
    psum = pools["psum"]
    for n in range(NL):
        for t in range(N_HT):
            r0 = t * HT
            idx = n * N_HT + t
            for cob in range(CB):
                pt = psum.tile([128, FREE], F32, tag="pt", name="pt")
                k = 0
                for dy in range(3):
                    for dx in range(3):
                        off = (r0 + dy) * PITCH + dx
                        nc.tensor.matmul(
                            pt[:],
                            wt[:, dy, dx, :, cob * 128:(cob + 1) * 128],
                            xb[:, n, :, off:off + FREE],
                            start=(k == 0),
                            stop=(k == 8),
                            perf_mode=DR,
                        )
                        k += 1
                ptv = pt[:].rearrange("p (a b) -> p a b", b=PITCH)
                ysl = y16[:, cob, n, r0:r0 + HT, :]
                # fp32 PSUM -> exact integers in fp16 (junk pitch cols dropped)
                copy_eng(ysl, ptv[:, :, 0:W])
                # one Welford record per tile (contiguous fp16 read)
                nc.vector.bn_stats(
                    out=recs[:, cob, idx, :],
                    in_=y16f[:, cob, n, r0 * W:(r0 + HT) * W])


def _stats_to_sums(nc, pools, recs, st):
    """bn_aggr per channel block, convert (mean, var) -> (sum, sumsq) local."""
    small = pools["small"]
    for cob in range(CB):
        mv = small.tile([128, 2], F32, tag=f"mv{cob}", name=f"mv{cob}")
        nc.vector.bn_aggr(out=mv[:], in_=recs[:, cob])
        msq = small.tile([128, 1], F32, tag=f"smsq{cob}", name=f"smsq{cob}")
        nc.vector.tensor_scalar_mul(st[:, 2 * cob:2 * cob + 1], mv[:, 0:1], M_LOC)
        nc.vector.tensor_mul(msq[:], mv[:, 0:1], mv[:, 0:1])
        nc.vector.tensor_add(msq[:], msq[:], mv[:, 1:2])
        nc.vector.tensor_scalar_mul(st[:, 2 * cob + 1:2 * cob + 2], msq[:], M_LOC)


def _bn_affine(nc, pools, gstats, gb, g_col, b_col, a_out, b_out):
    """Per-channel-block A/B:  A = g * rsqrt(var + eps_eff),  B = b - mean * A.

    gstats: [128, 4] globally-reduced (sum, sumsq) per channel block
    """
    small = pools["small"]
    epst = pools["epst"]
    for cob in range(CB):
        mean = small.tile([128, 1], F32, tag=f"mean{cob}", name=f"mean{cob}")
        ex2 = small.tile([128, 1], F32, tag=f"ex2{cob}", name=f"ex2{cob}")
        msq = small.tile([128, 1], F32, tag=f"msq{cob}", name=f"msq{cob}")
        var = small.tile([128, 1], F32, tag=f"var{cob}", name=f"var{cob}")
        rstd = small.tile([128, 1], F32, tag=f"rstd{cob}", name=f"rstd{cob}")
        nc.vector.tensor_scalar_mul(mean[:], gstats[:, 2 * cob:2 * cob + 1], 1.0 / M_TOT)
        nc.vector.tensor_scalar_mul(ex2[:], gstats[:, 2 * cob + 1:2 * cob + 2], 1.0 / M_TOT)
        # var = ex2 - mean^2
        nc.vector.tensor_mul(msq[:], mean[:], mean[:])
        nc.vector.tensor_sub(var[:], ex2[:], msq[:])
        # rstd = 1 / sqrt(var + eps_eff)
        nc.scalar.activation(out=rstd[:], in_=var[:], func=AF.Sqrt, bias=epst[:])
        nc.vector.reciprocal(rstd[:], rstd[:])
        # A = g * rstd ; B = b - mean * A
        nc.vector.tensor_mul(a_out[cob][:], gb[:, g_col + cob:g_col + cob + 1], rstd[:])
        nc.vector.tensor_mul(mean[:], mean[:], a_out[cob][:])
        nc.vector.tensor_sub(b_out[cob][:], gb[:, b_col + cob:b_col + cob + 1], mean[:])


def _allreduce_sums(nc, pools, st, tag):
    """AllReduce [128, 4] local sums -> [128, 4] global sums in SBUF."""
    small = pools["small"]
    dram = pools["dram"]
    rg = pools["rg"]
    d_in = dram.tile([128, 4], F32, tag=f"din{tag}", name=f"din{tag}")
    d_out = dram.tile([128, 4], F32, tag=f"dout{tag}", name=f"dout{tag}")
    nc.sync.dma_start(out=d_in[:], in_=st[:])
    nc.gpsimd.collective_compute(
        "AllReduce", ALU.add, replica_groups=rg,
        ins=[d_in.opt()], outs=[d_out.opt()],
    )
    gst = small.tile([128, 4], F32, tag=f"gst{tag}", name=f"gst{tag}")
    nc.sync.dma_start(out=gst[:], in_=d_out[:])
    return gst


def build():
    """Build + compile the per-core Bass program (SPMD, 8 cores)."""
    nc = bacc.Bacc("TRN2", target_bir_lowering=False, debug=False,
                   num_devices=N_CORES)

    x_in = nc.dram_tensor("x", [NL, C, H, W], F32, kind="ExternalInput").ap()
    w1_in = nc.dram_tensor("w1t", [3, 3, 128, 2, C], FP8, kind="ExternalInput").ap()
    w2_in = nc.dram_tensor("w2t", [3, 3, 128, 2, C], FP8, kind="ExternalInput").ap()
    gb_in = nc.dram_tensor("gb", [128, 8], F32, kind="ExternalInput").ap()
    out_d = nc.dram_tensor("out", [NL, C, H, W], F32, kind="ExternalOutput").ap()

    rg = [list(range(N_CORES))]

    with tile.TileContext(nc) as tc:
        import contextlib
        with contextlib.ExitStack() as ctx:
            consts = ctx.enter_context(tc.tile_pool(name="consts", bufs=1))
            xbp = ctx.enter_context(tc.tile_pool(name="xbp", bufs=1))
            x16p = ctx.enter_context(tc.tile_pool(name="x16p", bufs=1))
            y16p = ctx.enter_context(tc.tile_pool(name="y16p", bufs=1))
            statp = ctx.enter_context(tc.tile_pool(name="statp", bufs=1))
            small = ctx.enter_context(tc.tile_pool(name="small", bufs=1))
            psum = ctx.enter_context(tc.tile_pool(name="psum", bufs=6, space="PSUM"))
            dram = ctx.enter_context(tc.tile_pool(name="dram", bufs=1, space="DRAM"))
            epst = small.tile([128, 1], F32, tag="epst", name="epst")
            nc.vector.memset(epst[:], EPS_EFF)
            pools = {"psum": psum, "small": small, "epst": epst,
                     "dram": dram, "rg": rg}

            # ---- dummy AllReduce: absorb first-collective setup cost
            dzero = small.tile([128, 1], F32, tag="dzero", name="dzero")
            nc.vector.memset(dzero[:], 0.0)
            d_in0 = dram.tile([128, 1], F32, tag="d_in0", name="d_in0")
            d_out0 = dram.tile([128, 1], F32, tag="d_out0", name="d_out0")
            nc.sync.dma_start(out=d_in0[:], in_=dzero[:])
            nc.gpsimd.collective_compute(
                "AllReduce", ALU.add, replica_groups=rg,
                ins=[d_in0.opt()], outs=[d_out0.opt()],
            )

            # ---- constants (w1 in its own pool, released after conv1)
            w1p = tc.tile_pool(name="w1p", bufs=1)
            w1pp = w1p.__enter__()
            w1t = w1pp.tile([128, 3, 3, 2, C], FP8, tag="w1t", name="w1t")
            nc.gpsimd.dma_start(
                out=w1t[:],
                in_=w1_in[:].rearrange("dy dx ci two co -> ci dy dx two co"))
            w2t = consts.tile([128, 3, 3, 2, C], FP8, tag="w2t", name="w2t")
            nc.gpsimd.dma_start(
                out=w2t[:],
                in_=w2_in[:].rearrange("dy dx ci two co -> ci dy dx two co"))
            gb = consts.tile([128, 8], F32, tag="gb", name="gb")
            nc.gpsimd.dma_start(out=gb[:], in_=gb_in[:])

            # ---- padded binarized activations (conv1 input, then conv2
            # input).  58-elem row pitch; per-image sub-ranges keep the
            # dependency tracker fine-grained so conv1 starts after image 0.
            xb = xbp.tile([128, NL, CB, IMG], FP8, tag="xb", name="xb")
            # zero the halo borders + tail padding once
            xr = xb[:]  # [128, NL, CB, IMG]
            nc.vector.memset(xr[:, :, :, 0:PITCH], 0.0)               # top row
            nc.vector.memset(xr[:, :, :, IMG_RAW - PITCH:IMG], 0.0)   # bottom+pad
            xg = xr[:, :, :, 0:IMG_RAW].rearrange(
                "p n c (r w) -> p (n c) r w", w=PITCH)
            nc.vector.memset(xg[:, :, 1:HP - 1, 0:1], 0.0)            # left col
            nc.vector.memset(xg[:, :, 1:HP - 1, PITCH - 1:PITCH], 0.0)  # right col
            xbi = xg.rearrange("p (n c) r w -> p n c r w", n=NL)

            # ---- residual copy of x in fp16 (no second HBM read at the tail)
            x16 = x16p.tile([128, CB, NL, PIX], F16, tag="x16", name="x16")

            # ---- conv outputs as exact integers (reused for conv1 then conv2)
            y16 = y16p.tile([128, CB, NL, H, W], F16, tag="y16", name="y16")
            y16f = y16[:].rearrange("p c n a b -> p c n (a b)")
            y16v = y16[:]

            # ---- bn_stats records
            r1c = statp.tile([128, CB, NL * N_HT, 6], F32, tag="r1c", name="r1c")
            r2c = statp.tile([128, CB, NL * N_HT, 6], F32, tag="r2c", name="r2c")

            def copy_scalar(dst, src):
                nc.scalar.activation(out=dst, in_=src, func=AF.Copy)

            def copy_vector(dst, src):
                nc.vector.tensor_copy(dst, src)

            # ---- phase 0: all input DMAs up front (SP queue, HW-parallel)
            with tc.tile_pool(name="stage", bufs=4) as stage:
                xstages = []
                for n in range(NL):
                    for cib in range(CB):
                        xs = stage.tile([128, H, W], F32, tag="xstage",
                                        name="xstage")
                        nc.sync.dma_start(
                            out=xs[:], in_=x_in[n, cib * 128:(cib + 1) * 128, :, :])
                        xstages.append(xs)

                # ---- per image: binarize (scalar), conv1 tiles (PE), PSUM
                # copies + stats (vector); the fp16 residual cast rides on
                # vector after each image's conv work is queued.
                for n in range(NL):
                    for cib in range(CB):
                        xs = xstages[n * CB + cib]
                        nc.scalar.activation(
                            out=xbi[:, n, cib, 1:H + 1, 1:W + 1], in_=xs[:],
                            func=AF.Sign)
                    for t in range(N_HT):
                        r0 = t * HT
                        idx = n * N_HT + t
                        for cob in range(CB):
                            pt = psum.tile([128, FREE], F32, tag="pt", name="pt")
                            k = 0
                            for dy in range(3):
                                for dx in range(3):
                                    off = (r0 + dy) * PITCH + dx
                                    nc.tensor.matmul(
                                        pt[:],
                                        w1t[:, dy, dx, :, cob * 128:(cob + 1) * 128],
                                        xb[:, n, :, off:off + FREE],
                                        start=(k == 0),
                                        stop=(k == 8),
                                        perf_mode=DR,
                                    )
                                    k += 1
                            ptv = pt[:].rearrange("p (a b) -> p a b", b=PITCH)
                            nc.vector.tensor_copy(
                                y16v[:, cob, n, r0:r0 + HT, :], ptv[:, :, 0:W])
                            nc.vector.bn_stats(
                                out=r1c[:, cob, idx, :],
                                in_=y16f[:, cob, n, r0 * W:(r0 + HT) * W])
                    for cib in range(CB):
                        xs = xstages[n * CB + cib]
                        nc.vector.tensor_copy(
                            x16[:, cib, n, :],
                            xs[:].rearrange("p a b -> p (a b)"))
            w1p.__exit__(None, None, None)

            # ---- aggregate + AllReduce stats 1 -> BN1 affine
            st1 = small.tile([128, 4], F32, tag="st1", name="st1")
            _stats_to_sums(nc, pools, r1c[:], st1)
            gst1 = _allreduce_sums(nc, pools, st1, "1")
            a1 = [small.tile([128, 1], F32, tag=f"a1_{c}", name=f"a1_{c}") for c in range(CB)]
            b1 = [small.tile([128, 1], F32, tag=f"b1_{c}", name=f"b1_{c}") for c in range(CB)]
            _bn_affine(nc, pools, gst1, gb, g_col=0, b_col=2, a_out=a1, b_out=b1)

            # ---- phase 2: out1 = sign(A1 * y1 + B1) into the padded buffers
            for n in range(NL):
                for t in range(N_HT):
                    r0 = t * HT
                    for cob in range(CB):
                        nc.scalar.activation(
                            out=xbi[:, n, cob, r0 + 1:r0 + HT + 1, 1:W + 1],
                            in_=y16v[:, cob, n, r0:r0 + HT, :],
                            func=AF.Sign,
                            scale=a1[cob][:],
                            bias=b1[cob][:],
                        )

            # ---- conv2 + stats (y16 overwritten with conv2 integer sums;
            # ScalarE is busy with the phase-2 signs, PSUM copies on vector)
            _conv_phase(nc, pools, xb[:], w2t, y16v, y16f, r2c[:], copy_vector)

            # ---- aggregate + AllReduce stats 2 -> BN2 affine
            st2 = small.tile([128, 4], F32, tag="st2", name="st2")
            _stats_to_sums(nc, pools, r2c[:], st2)
            gst2 = _allreduce_sums(nc, pools, st2, "2")
            a2 = [small.tile([128, 1], F32, tag=f"a2_{c}", name=f"a2_{c}") for c in range(CB)]
            b2 = [small.tile([128, 1], F32, tag=f"b2_{c}", name=f"b2_{c}") for c in range(CB)]
            _bn_affine(nc, pools, gst2, gb, g_col=4, b_col=6, a_out=a2, b_out=b2)

            # ---- final: out = clip(A2 * y2 + B2 + x, -1, 1), pipelined with
            # the output DMA.  Clips alternate vector/gpsimd.
            youtp = ctx.enter_context(tc.tile_pool(name="youtp", bufs=3))
            for n in range(NL):
                for cib in range(CB):
                    yout = youtp.tile([128, PIX], F32, tag="yout", name="yout")
                    nc.scalar.activation(
                        out=yout[:], in_=y16f[:, cib, n, :], func=AF.Identity,
                        scale=a2[cib][:], bias=b2[cib][:])
                    nc.vector.tensor_add(yout[:], yout[:], x16[:, cib, n, :])
                    clip_eng = nc.vector if (n * CB + cib) % 2 == 0 else nc.gpsimd
                    clip_eng.tensor_scalar(
                        out=yout[:], in0=yout[:], scalar1=1.0, scalar2=-1.0,
                        op0=ALU.min, op1=ALU.max)
                    nc.sync.dma_start(
                        out=out_d[n, cib * 128:(cib + 1) * 128, :, :],
                        in_=yout[:].rearrange("p (a b) -> p a b", b=W))

    nc.compile()
    return nc


def _prep_inputs(x, w1, g1, b1, w2, g2, b2):
    """Host-side sharding + weight layout. Returns per-core input maps."""
    x = np.ascontiguousarray(np.asarray(x, dtype=np.float32))

    # sign(w) in [dy, dx, ci%128, ci//128, co] fp8 layout; +-1/0 exact
    def prep_w(w):
        wt = np.sign(np.asarray(w, np.float32)).transpose(2, 3, 1, 0)  # dy dx ci co
        wt = wt.reshape(3, 3, 2, 128, C).transpose(0, 1, 3, 2, 4)      # dy dx 128 2 co
        return np.ascontiguousarray(wt).astype(NP_FP8)

    w1t = prep_w(w1)
    w2t = prep_w(w2)
    gb = np.stack(
        [np.asarray(v, np.float32)[c * 128:(c + 1) * 128]
         for v in (g1, b1, g2, b2) for c in range(CB)],
        axis=1,
    )
    # column order: g1_0 g1_1 b1_0 b1_1 g2_0 g2_1 b2_0 b2_1
    gb = np.ascontiguousarray(gb)
    in_maps = []
    for c in range(N_CORES):
        in_maps.append({
            "x": x[c * NL:(c + 1) * NL],
            "w1t": w1t,
            "w2t": w2t,
            "gb": gb,
        })
    return in_maps


def run(inputs, trace=False):
    """Run the kernel on 8 cores; returns (full_output, BassKernelResults)."""
    if "nc" not in _CACHE:
        _CACHE["nc"] = build()
    nc = _CACHE["nc"]
    in_maps = _prep_inputs(**inputs)
    res = bass_utils.run_bass_kernel_spmd(
        nc, in_maps, core_ids=list(range(N_CORES)), trace=trace)
    out = np.concatenate([res.results[c]["out"] for c in range(N_CORES)], axis=0)
    return out, res


def kernel(**inputs):
    out, _ = run(inputs, trace=False)
    return out


# revision 4
# speedup vs baseline: 1.2156x; 1.0547x over previous
"""Trainium2 Bass kernel for a binarized (1w1a) BasicBlock:

    out1 = hardtanh(BN1(binconv(x, w1)))          # BN in training mode (batch stats)
    out  = hardtanh(BN2(binconv(out1, w2)) + x)   # identity shortcut

binconv(x, w) = conv3x3(sign(x), sign(w), pad=1) * (SCALE / K)

Sharding: data-parallel over batch (4 images per core on 8 cores), weights
replicated.  BN batch statistics (per-channel sum and sum-of-squares) are
combined with a tiny cross-core AllReduce.

v2 design notes (evolved from the 64-pitch baseline):
  - sign() values (+-1, 0) are exact in fp8, and the 3x3x256 conv accumulates
    integers |S| <= 2304 in fp32 PSUM, so the convolutions are bit-exact at
    fp8 DoubleRow rate.  SCALE/K commutes through BatchNorm and is folded
    into eps: eps_eff = eps / (SCALE/K)^2.
  - conv3x3 = 9 shifted matmuls accumulated in PSUM.  Activations live in
    SBUF as [128, n, cb, 58*58(+12 pad)] zero-haloed images with a 58-elem
    row pitch; every tap is a plain access-pattern offset (HW-verified that
    DoubleRow tolerates unaligned rhs offsets, so no pitch-64 padding and no
    PSUM-window shifting is needed).  Each matmul covers 8 output rows =
    464 streamed columns of which 448 are useful (96.6%).
  - LDWEIGHTS is hidden under MATMUL by the PE's reorder window, so the
    matmul stream runs at ~N/2.4GHz per tap.
  - x is kept resident in SBUF as fp16 for the residual add: no second
    HBM read of x, and the tail is a compute/DMA pipeline.
  - BN stats ride on VectorE (bn_stats per conv tile) while TensorE streams;
    the cross-core exchange is a single small AllReduce per BN.
  - Engine budget: ScalarE does input signs, then the 56 phase-2
    sign(A*y+B) re-binarizations, then the tail affine; VectorE does the
    PSUM->f16 copies, bn_stats, x16 casts, residual adds; GpSimd takes half
    the tail clips; SP queues all data DMA.
"""

import numpy as np
import ml_dtypes

import concourse.bass as bass
import concourse.tile as tile
from concourse import bacc, mybir
from concourse import bass_utils

N_CORES = 8
N, C, H, W = 32, 256, 56, 56
NL = N // N_CORES          # images per core (4)
CB = C // 128              # channel blocks (2)
PITCH = 58                 # padded row pitch (1 halo + 56 + 1 halo)
HP = 58                    # padded rows
IMG_RAW = HP * PITCH       # 3364
IMG = 3376                 # padded to 16B multiple (DoubleRow block stride)
HT = 8                     # output rows per tile
N_HT = H // HT             # 7 tiles per image
FREE = HT * PITCH          # 464 streamed columns per matmul
PIX = H * W                # 3136
SCALE, K = 1.0, 2
EPS = 1e-5
ALPHA = SCALE / K
EPS_EFF = EPS / (ALPHA * ALPHA)
M_TOT = float(N * H * W)   # BN reduction count (global batch)
M_LOC = float(NL * H * W)

F32 = mybir.dt.float32
F16 = mybir.dt.float16
FP8 = mybir.dt.float8e4
NP_FP8 = ml_dtypes.float8_e4m3
AF = mybir.ActivationFunctionType
ALU = mybir.AluOpType
DR = mybir.MatmulPerfMode.DoubleRow

_CACHE = {}


def _conv_phase(nc, pools, xb, wt, y16, y16f, recs, copy_eng):
    """One binarized conv3x3 over all local images + stats accumulation.

    xb:   [128, NL, CB, IMG] fp8 padded/binarized inputs
    wt:   [128, 3, 3, 2, C] fp8 weights ([ci, dy, dx, ci_blk, co] layout)
    y16:  [128, CB, NL, H, W] f16 conv outputs (exact integer sums)
    y16f: [128, CB, NL, PIX] flat view of the same tile
    recs: [128, CB, NL * N_HT, 6] f32 bn_stats records (equal-count tiles,
          as bn_aggr's equal-weight merge requires)
    """
    psum = pools["psum"]
    for n in range(NL):
        for t in range(N_HT):
            r0 = t * HT
            idx = n * N_HT + t
            for cob in range(CB):
                pt = psum.tile([128, FREE], F32, tag="pt", name="pt")
                k = 0
                for dy in range(3):
                    for dx in range(3):
                        off = (r0 + dy) * PITCH + dx
                        nc.tensor.matmul(
                            pt[:],
                            wt[:, dy, dx, :, cob * 128:(cob + 1) * 128],
                            xb[:, n, :, off:off + FREE],
                            start=(k == 0),
                            stop=(k == 8),
                            perf_mode=DR,
                        )
                        k += 1
                ptv = pt[:].rearrange("p (a b) -> p a b", b=PITCH)
                ysl = y16[:, cob, n, r0:r0 + HT, :]
                # fp32 PSUM -> exact integers in fp16 (junk pitch cols dropped)
                copy_eng(ysl, ptv[:, :, 0:W])
                # one Welford record per tile (contiguous fp16 read)
                nc.vector.bn_stats(
                    out=recs[:, cob, idx, :],
                    in_=y16f[:, cob, n, r0 * W:(r0 + HT) * W])


def _stats_to_sums(nc, pools, recs, st):
    """bn_aggr per channel block, convert (mean, var) -> (sum, sumsq) local."""
    small = pools["small"]
    for cob in range(CB):
        mv = small.tile([128, 2], F32, tag=f"mv{cob}", name=f"mv{cob}")
        nc.vector.bn_aggr(out=mv[:], in_=recs[:, cob])
        msq = small.tile([128, 1], F32, tag=f"smsq{cob}", name=f"smsq{cob}")
        nc.vector.tensor_scalar_mul(st[:, 2 * cob:2 * cob + 1], mv[:, 0:1], M_LOC)
        nc.vector.tensor_mul(msq[:], mv[:, 0:1], mv[:, 0:1])
        nc.vector.tensor_add(msq[:], msq[:], mv[:, 1:2])
        nc.vector.tensor_scalar_mul(st[:, 2 * cob + 1:2 * cob + 2], msq[:], M_LOC)


def _bn_affine(nc, pools, gstats, gb, g_col, b_col, a_out, b_out):
    """Per-channel-block A/B:  A = g * rsqrt(var + eps_eff),  B = b - mean * A.

    gstats: [128, 4] globally-reduced (sum, sumsq) per channel block
    """
    small = pools["small"]
    epst = pools["epst"]
    for cob in range(CB):
        mean = small.tile([128, 1], F32, tag=f"mean{cob}", name=f"mean{cob}")
        ex2 = small.tile([128, 1], F32, tag=f"ex2{cob}", name=f"ex2{cob}")
        msq = small.tile([128, 1], F32, tag=f"msq{cob}", name=f"msq{cob}")
        var = small.tile([128, 1], F32, tag=f"var{cob}", name=f"var{cob}")
        rstd = small.tile([128, 1], F32, tag=f"rstd{cob}", name=f"rstd{cob}")
        nc.vector.tensor_scalar_mul(mean[:], gstats[:, 2 * cob:2 * cob + 1], 1.0 / M_TOT)
        nc.vector.tensor_scalar_mul(ex2[:], gstats[:, 2 * cob + 1:2 * cob + 2], 1.0 / M_TOT)
        # var = ex2 - mean^2
        nc.vector.tensor_mul(msq[:], mean[:], mean[:])
        nc.vector.tensor_sub(var[:], ex2[:], msq[:])
        # rstd = 1 / sqrt(var + eps_eff)
        nc.scalar.activation(out=rstd[:], in_=var[:], func=AF.Sqrt, bias=epst[:])
        nc.vector.reciprocal(rstd[:], rstd[:])
        # A = g * rstd ; B = b - mean * A
        nc.vector.tensor_mul(a_out[cob][:], gb[:, g_col + cob:g_col + cob + 1], rstd[:])
        nc.vector.tensor_mul(mean[:], mean[:], a_out[cob][:])
        nc.vector.tensor_sub(b_out[cob][:], gb[:, b_col + cob:b_col + cob + 1], mean[:])


def _allreduce_sums(nc, pools, st, tag):
    """AllReduce [128, 4] local sums -> [128, 4] global sums in SBUF."""
    small = pools["small"]
    dram = pools["dram"]
    rg = pools["rg"]
    d_in = dram.tile([128, 4], F32, tag=f"din{tag}", name=f"din{tag}")
    d_out = dram.tile([128, 4], F32, tag=f"dout{tag}", name=f"dout{tag}")
    nc.sync.dma_start(out=d_in[:], in_=st[:])
    nc.gpsimd.collective_compute(
        "AllReduce", ALU.add, replica_groups=rg,
        ins=[d_in.opt()], outs=[d_out.opt()],
    )
    gst = small.tile([128, 4], F32, tag=f"gst{tag}", name=f"gst{tag}")
    nc.sync.dma_start(out=gst[:], in_=d_out[:])
    return gst


def build():
    """Build + compile the per-core Bass program (SPMD, 8 cores)."""
    nc = bacc.Bacc("TRN2", target_bir_lowering=False, debug=False,
                   num_devices=N_CORES)

    x_in = nc.dram_tensor("x", [NL, C, H, W], F32, kind="ExternalInput").ap()
    w1_in = nc.dram_tensor("w1t", [3, 3, 128, 2, C], FP8, kind="ExternalInput").ap()
    w2_in = nc.dram_tensor("w2t", [3, 3, 128, 2, C], FP8, kind="ExternalInput").ap()
    gb_in = nc.dram_tensor("gb", [128, 8], F32, kind="ExternalInput").ap()
    out_d = nc.dram_tensor("out", [NL, C, H, W], F32, kind="ExternalOutput").ap()

    rg = [list(range(N_CORES))]

    with tile.TileContext(nc) as tc:
        import contextlib
        with contextlib.ExitStack() as ctx:
            consts = ctx.enter_context(tc.tile_pool(name="consts", bufs=1))
            xbp = ctx.enter_context(tc.tile_pool(name="xbp", bufs=1))
            x16p = ctx.enter_context(tc.tile_pool(name="x16p", bufs=1))
            y16p = ctx.enter_context(tc.tile_pool(name="y16p", bufs=1))
            statp = ctx.enter_context(tc.tile_pool(name="statp", bufs=1))
            small = ctx.enter_context(tc.tile_pool(name="small", bufs=1))
            psum = ctx.enter_context(tc.tile_pool(name="psum", bufs=6, space="PSUM"))
            dram = ctx.enter_context(tc.tile_pool(name="dram", bufs=1, space="DRAM"))
            epst = small.tile([128, 1], F32, tag="epst", name="epst")
            nc.vector.memset(epst[:], EPS_EFF)
            pools = {"psum": psum, "small": small, "epst": epst,
                     "dram": dram, "rg": rg}

            # ---- dummy AllReduce: absorb first-collective setup cost
            dzero = small.tile([128, 1], F32, tag="dzero", name="dzero")
            nc.vector.memset(dzero[:], 0.0)
            d_in0 = dram.tile([128, 1], F32, tag="d_in0", name="d_in0")
            d_out0 = dram.tile([128, 1], F32, tag="d_out0", name="d_out0")
            nc.sync.dma_start(out=d_in0[:], in_=dzero[:])
            nc.gpsimd.collective_compute(
                "AllReduce", ALU.add, replica_groups=rg,
                ins=[d_in0.opt()], outs=[d_out0.opt()],
            )

            # ---- constants (w1 in its own pool, released after conv1)
            w1p = tc.tile_pool(name="w1p", bufs=1)
            w1pp = w1p.__enter__()
            w1t = w1pp.tile([128, 3, 3, 2, C], FP8, tag="w1t", name="w1t")
            nc.gpsimd.dma_start(
                out=w1t[:],
                in_=w1_in[:].rearrange("dy dx ci two co -> ci dy dx two co"))
            w2t = consts.tile([128, 3, 3, 2, C], FP8, tag="w2t", name="w2t")
            nc.gpsimd.dma_start(
                out=w2t[:],
                in_=w2_in[:].rearrange("dy dx ci two co -> ci dy dx two co"))
            gb = consts.tile([128, 8], F32, tag="gb", name="gb")
            nc.gpsimd.dma_start(out=gb[:], in_=gb_in[:])

            # ---- padded binarized activations (conv1 input, then conv2
            # input).  58-elem row pitch; per-image sub-ranges keep the
            # dependency tracker fine-grained so conv1 starts after image 0.
            xb = xbp.tile([128, NL, CB, IMG], FP8, tag="xb", name="xb")
            # zero the halo borders + tail padding once
            xr = xb[:]  # [128, NL, CB, IMG]
            nc.vector.memset(xr[:, :, :, 0:PITCH], 0.0)               # top row
            nc.vector.memset(xr[:, :, :, IMG_RAW - PITCH:IMG], 0.0)   # bottom+pad
            xg = xr[:, :, :, 0:IMG_RAW].rearrange(
                "p n c (r w) -> p (n c) r w", w=PITCH)
            nc.vector.memset(xg[:, :, 1:HP - 1, 0:1], 0.0)            # left col
            nc.vector.memset(xg[:, :, 1:HP - 1, PITCH - 1:PITCH], 0.0)  # right col
            xbi = xg.rearrange("p (n c) r w -> p n c r w", n=NL)

            # ---- residual copy of x in fp16 (no second HBM read at the tail)
            x16 = x16p.tile([128, CB, NL, PIX], F16, tag="x16", name="x16")

            # ---- conv outputs as exact integers (reused for conv1 then conv2)
            y16 = y16p.tile([128, CB, NL, H, W], F16, tag="y16", name="y16")
            y16f = y16[:].rearrange("p c n a b -> p c n (a b)")
            y16v = y16[:]

            # ---- bn_stats records
            r1c = statp.tile([128, CB, NL * N_HT, 6], F32, tag="r1c", name="r1c")
            r2c = statp.tile([128, CB, NL * N_HT, 6], F32, tag="r2c", name="r2c")

            def copy_scalar(dst, src):
                nc.scalar.activation(out=dst, in_=src, func=AF.Copy)

            def copy_vector(dst, src):
                nc.vector.tensor_copy(dst, src)

            # ---- phase 0: staged input loads in half-image chunks so the
            # first conv tiles can start ~15us in.  bufs=3 throttles DMA
            # concurrency (the HW queues round-robin all active transfers,
            # so unbounded concurrency makes the FIRST image finish last).
            HH = H // 2  # 28 rows per half
            with tc.tile_pool(name="stage", bufs=3) as stage:
                xstages = {}
                for n in range(NL):
                    for half in range(2):
                        for cib in range(CB):
                            xs = stage.tile([128, HH, W], F32, tag="xstage",
                                            name="xstage")
                            nc.sync.dma_start(
                                out=xs[:],
                                in_=x_in[n, cib * 128:(cib + 1) * 128,
                                         half * HH:(half + 1) * HH, :])
                            xstages[(n, half, cib)] = xs

                # ---- per image: binarize halves (scalar), conv1 tiles (PE),
                # PSUM copies + stats (vector); the fp16 residual casts ride
                # on scalar after each image's signs.
                for n in range(NL):
                    for half in range(2):
                        for cib in range(CB):
                            xs = xstages[(n, half, cib)]
                            r0 = half * HH
                            nc.scalar.activation(
                                out=xbi[:, n, cib, r0 + 1:r0 + HH + 1, 1:W + 1],
                                in_=xs[:], func=AF.Sign)
                    for t in range(N_HT):
                        r0 = t * HT
                        idx = n * N_HT + t
                        for cob in range(CB):
                            pt = psum.tile([128, FREE], F32, tag="pt", name="pt")
                            k = 0
                            for dy in range(3):
                                for dx in range(3):
                                    off = (r0 + dy) * PITCH + dx
                                    nc.tensor.matmul(
                                        pt[:],
                                        w1t[:, dy, dx, :, cob * 128:(cob + 1) * 128],
                                        xb[:, n, :, off:off + FREE],
                                        start=(k == 0),
                                        stop=(k == 8),
                                        perf_mode=DR,
                                    )
                                    k += 1
                            ptv = pt[:].rearrange("p (a b) -> p a b", b=PITCH)
                            nc.vector.tensor_copy(
                                y16v[:, cob, n, r0:r0 + HT, :], ptv[:, :, 0:W])
                            nc.vector.bn_stats(
                                out=r1c[:, cob, idx, :],
                                in_=y16f[:, cob, n, r0 * W:(r0 + HT) * W])
                    for half in range(2):
                        for cib in range(CB):
                            xs = xstages[(n, half, cib)]
                            nc.scalar.activation(
                                out=x16[:, cib, n, half * HH * W:(half + 1) * HH * W],
                                in_=xs[:].rearrange("p a b -> p (a b)"),
                                func=AF.Copy)
            w1p.__exit__(None, None, None)

            # ---- aggregate + AllReduce stats 1 -> BN1 affine
            st1 = small.tile([128, 4], F32, tag="st1", name="st1")
            _stats_to_sums(nc, pools, r1c[:], st1)
            gst1 = _allreduce_sums(nc, pools, st1, "1")
            a1 = [small.tile([128, 1], F32, tag=f"a1_{c}", name=f"a1_{c}") for c in range(CB)]
            b1 = [small.tile([128, 1], F32, tag=f"b1_{c}", name=f"b1_{c}") for c in range(CB)]
            _bn_affine(nc, pools, gst1, gb, g_col=0, b_col=2, a_out=a1, b_out=b1)

            # ---- phase 2: out1 = sign(A1 * y1 + B1) into the padded buffers
            for n in range(NL):
                for t in range(N_HT):
                    r0 = t * HT
                    for cob in range(CB):
                        nc.scalar.activation(
                            out=xbi[:, n, cob, r0 + 1:r0 + HT + 1, 1:W + 1],
                            in_=y16v[:, cob, n, r0:r0 + HT, :],
                            func=AF.Sign,
                            scale=a1[cob][:],
                            bias=b1[cob][:],
                        )

            # ---- conv2 + stats (y16 overwritten with conv2 integer sums;
            # ScalarE is busy with the phase-2 signs, PSUM copies on vector)
            _conv_phase(nc, pools, xb[:], w2t, y16v, y16f, r2c[:], copy_vector)

            # ---- aggregate + AllReduce stats 2 -> BN2 affine
            st2 = small.tile([128, 4], F32, tag="st2", name="st2")
            _stats_to_sums(nc, pools, r2c[:], st2)
            gst2 = _allreduce_sums(nc, pools, st2, "2")
            a2 = [small.tile([128, 1], F32, tag=f"a2_{c}", name=f"a2_{c}") for c in range(CB)]
            b2 = [small.tile([128, 1], F32, tag=f"b2_{c}", name=f"b2_{c}") for c in range(CB)]
            _bn_affine(nc, pools, gst2, gb, g_col=4, b_col=6, a_out=a2, b_out=b2)

            # ---- final: out = clip(A2 * y2 + B2 + x, -1, 1), pipelined with
            # the output DMA.  The affine+add run in fp16 (2x DVE rate; the
            # values are O(1) so fp16 rounding is ~5e-4 abs, well inside
            # tolerance); the clip converts to the f32 output and alternates
            # vector/gpsimd so the DMA stays the bottleneck.
            youtp = ctx.enter_context(tc.tile_pool(name="youtp", bufs=3))
            y16tp = ctx.enter_context(tc.tile_pool(name="y16tp", bufs=3))
            for n in range(NL):
                for cib in range(CB):
                    yt = y16tp.tile([128, PIX], F16, tag="yt", name="yt")
                    nc.scalar.activation(
                        out=yt[:], in_=y16f[:, cib, n, :], func=AF.Identity,
                        scale=a2[cib][:], bias=b2[cib][:])
                    nc.vector.tensor_add(yt[:], yt[:], x16[:, cib, n, :])
                    yout = youtp.tile([128, PIX], F32, tag="yout", name="yout")
                    clip_eng = nc.vector if (n * CB + cib) % 2 == 0 else nc.gpsimd
                    clip_eng.tensor_scalar(
                        out=yout[:], in0=yt[:], scalar1=1.0, scalar2=-1.0,
                        op0=ALU.min, op1=ALU.max)
                    nc.sync.dma_start(
                        out=out_d[n, cib * 128:(cib + 1) * 128, :, :],
                        in_=yout[:].rearrange("p (a b) -> p a b", b=W))

    nc.compile()
    return nc


def _prep_inputs(x, w1, g1, b1, w2, g2, b2):
    """Host-side sharding + weight layout. Returns per-core input maps."""
    x = np.ascontiguousarray(np.asarray(x, dtype=np.float32))

    # sign(w) in [dy, dx, ci%128, ci//128, co] fp8 layout; +-1/0 exact
    def prep_w(w):
        wt = np.sign(np.asarray(w, np.float32)).transpose(2, 3, 1, 0)  # dy dx ci co
        wt = wt.reshape(3, 3, 2, 128, C).transpose(0, 1, 3, 2, 4)      # dy dx 128 2 co
        return np.ascontiguousarray(wt).astype(NP_FP8)

    w1t = prep_w(w1)
    w2t = prep_w(w2)
    gb = np.stack(
        [np.asarray(v, np.float32)[c * 128:(c + 1) * 128]
         for v in (g1, b1, g2, b2) for c in range(CB)],
        axis=1,
    )
    # column order: g1_0 g1_1 b1_0 b1_1 g2_0 g2_1 b2_0 b2_1
    gb = np.ascontiguousarray(gb)
    in_maps = []
    for c in range(N_CORES):
        in_maps.append({
            "x": x[c * NL:(c + 1) * NL],
            "w1t": w1t,
            "w2t": w2t,
            "gb": gb,
        })
    return in_maps


def run(inputs, trace=False):
    """Run the kernel on 8 cores; returns (full_output, BassKernelResults)."""
    if "nc" not in _CACHE:
        _CACHE["nc"] = build()
    nc = _CACHE["nc"]
    in_maps = _prep_inputs(**inputs)
    res = bass_utils.run_bass_kernel_spmd(
        nc, in_maps, core_ids=list(range(N_CORES)), trace=trace)
    out = np.concatenate([res.results[c]["out"] for c in range(N_CORES)], axis=0)
    return out, res


def kernel(**inputs):
    out, _ = run(inputs, trace=False)
    return out
